# revision 7
# baseline (speedup 1.0000x reference)
"""DiceLoss kernel: fp8 inputs + DoubleRow PE matmuls via the diagonal trick.

x is quantized to fp8e4m3 on the host (halves HBM traffic; DMA is the
roofline at ~21.6us/core). Per class a bf16 mask (lab==c)*2^-15 is built
in one tensor_scalar (4x DVE mode for most classes, gpsimd for a few to
balance load); the bf16 tile's odd bytes, bitcast to fp8e4, are exactly
1.0/0.0 and feed the PE as the DoubleRow stationary at 0.5 cyc/row. Per
class the PE accumulates an intersect block (mask^T @ x) and a squares
block (x^T @ x) into adjacent 64-col PSUM regions at partition base 0
(the ISA rejects other dst partitions at 128-row contraction). Extraction
is pipelined across engines so DVE stays mask-bound: ACT copies the
128-col class slot PSUM->SBUF (gpsimd cannot read PSUM, and SBUF operands
give DVE its 2x mode), then two DVE ident-multiply STTs lagged three
classes behind write the diagonals' row-sums into stats[:, c]. The 32
128-col class slots tile PSUM exactly; class 32 reuses slot 0 long after
its extract. x loads are grouped 4 classes per DMA so HWDGE overhead
stays off the critical path. Host sums partials and applies the ratio.
"""
import numpy as np
import ml_dtypes
import concourse.bacc as bacc
import concourse.mybir as mybir
import concourse.tile as tile
from concourse.bass_utils import run_bass_kernel_spmd

N_CORES = 8
B, C, X, Y, Z = 2, 33, 96, 96, 96
XS = X // N_CORES
VOX = XS * Y * Z
P = 128
F = VOX // P
FB = B * F                   # 1728
KT = FB // 64                # 27 k-tiles of 64 cols
SMOOTH = 1e-5
MASK_LO = float(2.0 ** -15)  # bf16 0x3800: odd byte 0x38 == fp8e4 1.0
POOL_MASKS = {5, 10, 15, 20, 25, 29}

_cached = {}


def _build():
    nc = bacc.Bacc("TRN2", target_bir_lowering=False, debug=False,
                   num_devices=N_CORES)
    bf16 = mybir.dt.bfloat16
    fp8 = mybir.dt.float8e4
    f32 = mybir.dt.float32
    DR = mybir.MatmulPerfMode.DoubleRow
    x_in = nc.dram_tensor("x", [C, P, FB], fp8, kind="ExternalInput")
    lab_in = nc.dram_tensor("lab", [P, KT, 64], bf16, kind="ExternalInput")
    sel_in = nc.dram_tensor("sel", [64, 64], bf16, kind="ExternalInput")
    stats = nc.dram_tensor("stats", [P, C], f32, kind="ExternalOutput")
    with tile.TileContext(nc) as tc:
        with (
            tc.tile_pool(name="xp", bufs=1) as xp,
            tc.tile_pool(name="labp", bufs=1) as labp,
            tc.tile_pool(name="maskp", bufs=6) as maskp,
            tc.tile_pool(name="pmaskp", bufs=len(POOL_MASKS)) as pmaskp,
            tc.tile_pool(name="scr", bufs=12) as scrp,
            tc.tile_pool(name="stat", bufs=1) as statp,
            tc.tile_pool(name="psum", bufs=1, space="PSUM") as psp,
        ):
            lab_t = labp.tile([P, KT, 64], bf16)
            nc.sync.dma_start(lab_t[:], lab_in[:, :, :])
            sel_t = statp.tile([64, 64], bf16, tag="sel")
            nc.sync.dma_start(sel_t[:], sel_in[:, :])
            stat_t = statp.tile([P, C], f32, tag="stat")
            xt = xp.tile([P, C, KT, 64], fp8)
            groups = [(0, 1)] + [(1 + 4 * g, 4) for g in range(8)]
            for c0, n in groups:
                nc.sync.dma_start(xt[:, c0:c0 + n, :, :],
                                  x_in[c0:c0 + n, :, :])
            ps = psp.tile([P, 4096], f32)
            pending = []

            def emit_extract(c, scr):
                nc.vector.scalar_tensor_tensor(
                    out=scr[:, 0:64], in0=scr[:, 0:64], scalar=0.0,
                    in1=sel_t[:], op0=mybir.AluOpType.bypass,
                    op1=mybir.AluOpType.mult,
                    accum_out=stat_t[0:64, c:c + 1])
                nc.vector.scalar_tensor_tensor(
                    out=scr[:, 64:128], in0=scr[:, 64:128], scalar=0.0,
                    in1=sel_t[:], op0=mybir.AluOpType.bypass,
                    op1=mybir.AluOpType.mult,
                    accum_out=stat_t[64:128, c:c + 1])

            for c in range(C):
                if c in POOL_MASKS:
                    mt = pmaskp.tile([P, KT, 64], bf16)
                    eng = nc.gpsimd
                else:
                    mt = maskp.tile([P, KT, 64], bf16)
                    eng = nc.vector
                eng.tensor_scalar(mt[:], lab_t[:], float(c), MASK_LO,
                                  mybir.AluOpType.is_equal,
                                  mybir.AluOpType.mult)
                m8 = mt[:].bitcast(fp8)          # [P, KT, 128]; odd lanes
                xc = xt[:, c]                    # [P, KT, 64]
                o = (c % 32) * 128
                for j in range(13):
                    k = 2 * j
                    nc.tensor.matmul(
                        ps[0:64, o:o + 64], m8[:, k:k + 2, 1::2],
                        xc[:, k:k + 2, :], start=(j == 0), stop=False,
                        perf_mode=DR, skip_group_check=True)
                nc.tensor.matmul(
                    ps[0:64, o:o + 64], m8[:, 26:27, 1::2], xc[:, 26, :],
                    start=False, stop=True, skip_group_check=True)
                for j in range(13):
                    k = 2 * j
                    nc.tensor.matmul(
                        ps[0:64, o + 64:o + 128], xc[:, k:k + 2, :],
                        xc[:, k:k + 2, :], start=(j == 0), stop=False,
                        perf_mode=DR, skip_group_check=True)
                nc.tensor.matmul(
                    ps[0:64, o + 64:o + 128], xc[:, 26, :], xc[:, 26, :],
                    start=False, stop=True, skip_group_check=True)
                scr = scrp.tile([64, 128], bf16)
                nc.scalar.activation(
                    out=scr[:], in_=ps[0:64, o:o + 128],
                    func=mybir.ActivationFunctionType.Copy)
                pending.append((c, scr))
                if len(pending) > 8:
                    emit_extract(*pending.pop(0))
            for args in pending:
                emit_extract(*args)
            nc.sync.dma_start(stats[:, :], stat_t[:])
    nc.compile()
    return nc


def _get_nc():
    if "nc" not in _cached:
        _cached["nc"] = _build()
    return _cached["nc"]


def kernel(outputs, label):
    nc = _get_nc()
    outputs = np.asarray(outputs)
    lab_np = np.asarray(label)
    fp8 = ml_dtypes.float8_e4m3
    bf16 = ml_dtypes.bfloat16
    sel = np.eye(64, dtype=ml_dtypes.bfloat16)
    in_maps = []
    for k in range(N_CORES):
        xs = outputs[:, :, k * XS:(k + 1) * XS].reshape(B, C, P, F)
        xs = np.ascontiguousarray(xs.transpose(1, 2, 0, 3)).reshape(C, P, FB)
        ls = lab_np[:, k * XS:(k + 1) * XS].reshape(B, P, F)
        ls = np.ascontiguousarray(ls.transpose(1, 0, 2)).reshape(P, KT, 64)
        in_maps.append({"x": xs.astype(fp8), "lab": ls.astype(bf16),
                        "sel": sel})
    res = run_bass_kernel_spmd(nc, in_maps, core_ids=list(range(N_CORES)))
    intersect = np.zeros(C, np.float64)
    sumsq = np.zeros(C, np.float64)
    for r in res.results:
        st = r["stats"].astype(np.float64)
        intersect += st[:64].sum(axis=0)
        sumsq += st[64:].sum(axis=0)
    labels_sum = np.bincount(
        lab_np.reshape(-1).astype(np.int64), minlength=C).astype(np.float64)
    dice = (2.0 * intersect + SMOOTH) / (sumsq + labels_sum + SMOOTH)
    return np.float32(np.mean(1.0 - dice))


# revision 8
# speedup vs baseline: 1.0832x; 1.0832x over previous
"""DiceLoss kernel: fp8 inputs + DoubleRow PE matmuls via the diagonal trick.

x is quantized to fp8e4m3 on the host (halves HBM traffic; DMA is the
roofline at ~21.6us/core). Per class a bf16 mask (lab==c)*2^-15 is built
in one tensor_scalar (4x DVE mode for most classes, gpsimd for a few to
balance load); the bf16 tile's odd bytes, bitcast to fp8e4, are exactly
1.0/0.0 and feed the PE as the DoubleRow stationary at 0.5 cyc/row.
Columns are split into 14 k-tiles of 128 (1728 padded to 1792; pad label
cols are -1 and pad x cols are memset to 0 so both stats are unbiased),
so each stat needs just 7 DoubleRow matmuls -- PE sequencer issue rate,
not engine throughput, binds at finer tilings. Per class the PE
accumulates an intersect block (mask^T @ x) and a squares block (x^T @ x)
into an adjacent [128,128]-pair PSUM slot at partition base 0 (the ISA
rejects other dst partitions at 128-row contraction). Extraction is
pipelined across engines so DVE stays mask-bound: ACT copies the 256-col
class slot PSUM->SBUF as bf16 (gpsimd cannot read PSUM; all-SBUF bf16
operands give DVE its 4x mode), then two DVE ident-multiply STTs lagged
eight classes behind write the diagonals' row-sums into per-stat columns.
The 16 class slots are reused with period 16, long after each extract.
x loads are grouped 4 classes per DMA so HWDGE overhead stays off the
critical path. Host sums partials and applies the dice ratio.
"""
import numpy as np
import ml_dtypes
import concourse.bacc as bacc
import concourse.mybir as mybir
import concourse.tile as tile
from concourse.ap import AP
from concourse.bass_utils import run_bass_kernel_spmd

N_CORES = 8
B, C, X, Y, Z = 2, 33, 96, 96, 96
XS = X // N_CORES
VOX = XS * Y * Z
P = 128
F = VOX // P
FB = B * F                   # 1728
KT = 14                      # k-tiles of 128 cols (FB padded to 1792)
FBP = KT * 128
SMOOTH = 1e-5
MASK_LO = float(2.0 ** -15)  # bf16 0x3800: odd byte 0x38 == fp8e4 1.0
POOL_MASKS = {5, 10, 15, 20, 25, 29}

_cached = {}


def _flat(ap3, ncls):
    """[P, ncls*FB]-shaped DMA view (skipping pad cols) of x/lab tiles."""
    dims = [list(ap3.ap[0])] + ([[FBP, ncls]] if ncls > 1 else []) + [[1, FB]]
    return AP(ap3.tensor, ap3.offset, dims)


def _build():
    nc = bacc.Bacc("TRN2", target_bir_lowering=False, debug=False,
                   num_devices=N_CORES)
    bf16 = mybir.dt.bfloat16
    fp8 = mybir.dt.float8e4
    f32 = mybir.dt.float32
    DR = mybir.MatmulPerfMode.DoubleRow
    x_in = nc.dram_tensor("x", [C, P, FB], fp8, kind="ExternalInput")
    lab_in = nc.dram_tensor("lab", [P, FB], bf16, kind="ExternalInput")
    sel_in = nc.dram_tensor("sel", [P, P], bf16, kind="ExternalInput")
    stats = nc.dram_tensor("stats", [2, P, C], f32, kind="ExternalOutput")
    with tile.TileContext(nc) as tc:
        with (
            tc.tile_pool(name="xp", bufs=1) as xp,
            tc.tile_pool(name="labp", bufs=1) as labp,
            tc.tile_pool(name="maskp", bufs=6) as maskp,
            tc.tile_pool(name="pmaskp", bufs=len(POOL_MASKS)) as pmaskp,
            tc.tile_pool(name="scr", bufs=12) as scrp,
            tc.tile_pool(name="stat", bufs=1) as statp,
            tc.tile_pool(name="psum", bufs=1, space="PSUM") as psp,
        ):
            lab_t = labp.tile([P, KT, 128], bf16)
            nc.sync.dma_start(_flat(lab_t[:], 1), lab_in[:, :])
            nc.gpsimd.memset(lab_t[:, KT - 1, 64:128], -1.0)
            sel_t = statp.tile([P, P], bf16, tag="sel")
            nc.sync.dma_start(sel_t[:], sel_in[:, :])
            stat_i = statp.tile([P, C], f32, tag="sti")
            stat_s = statp.tile([P, C], f32, tag="sts")
            xt = xp.tile([P, C, KT, 128], fp8)
            nc.gpsimd.memset(xt[:, :, KT - 1, 64:128], 0.0)
            groups = [(0, 1)] + [(1 + 4 * g, 4) for g in range(8)]
            for c0, n in groups:
                nc.sync.dma_start(_flat(xt[:, c0:c0 + n], n),
                                  x_in[c0:c0 + n, :, :])
            ps = psp.tile([P, 4096], f32)
            pending = []

            def emit_extract(c, scr):
                nc.vector.scalar_tensor_tensor(
                    out=scr[:, 0:128], in0=scr[:, 0:128], scalar=0.0,
                    in1=sel_t[:], op0=mybir.AluOpType.bypass,
                    op1=mybir.AluOpType.mult,
                    accum_out=stat_i[:, c:c + 1])
                nc.vector.scalar_tensor_tensor(
                    out=scr[:, 128:256], in0=scr[:, 128:256], scalar=0.0,
                    in1=sel_t[:], op0=mybir.AluOpType.bypass,
                    op1=mybir.AluOpType.mult,
                    accum_out=stat_s[:, c:c + 1])

            for c in range(C):
                if c in POOL_MASKS:
                    mt = pmaskp.tile([P, KT, 128], bf16)
                    eng = nc.gpsimd
                else:
                    mt = maskp.tile([P, KT, 128], bf16)
                    eng = nc.vector
                eng.tensor_scalar(mt[:], lab_t[:], float(c), MASK_LO,
                                  mybir.AluOpType.is_equal,
                                  mybir.AluOpType.mult)
                m8 = mt[:].bitcast(fp8)          # [P, KT, 256]; odd lanes
                xc = xt[:, c]                    # [P, KT, 128]
                o = (c % 16) * 256
                for j in range(7):
                    k = 2 * j
                    nc.tensor.matmul(
                        ps[:, o:o + 128], m8[:, k:k + 2, 1::2],
                        xc[:, k:k + 2, :], start=(j == 0), stop=(j == 6),
                        perf_mode=DR, skip_group_check=True)
                for j in range(7):
                    k = 2 * j
                    nc.tensor.matmul(
                        ps[:, o + 128:o + 256], xc[:, k:k + 2, :],
                        xc[:, k:k + 2, :], start=(j == 0), stop=(j == 6),
                        perf_mode=DR, skip_group_check=True)
                scr = scrp.tile([P, 256], bf16)
                nc.scalar.activation(
                    out=scr[:], in_=ps[:, o:o + 256],
                    func=mybir.ActivationFunctionType.Copy)
                pending.append((c, scr))
                if len(pending) > 8:
                    emit_extract(*pending.pop(0))
            for args in pending:
                emit_extract(*args)
            nc.sync.dma_start(stats[0, :, :], stat_i[:])
            nc.sync.dma_start(stats[1, :, :], stat_s[:])
    nc.compile()
    return nc


def _get_nc():
    if "nc" not in _cached:
        _cached["nc"] = _build()
    return _cached["nc"]


def kernel(outputs, label):
    nc = _get_nc()
    outputs = np.asarray(outputs)
    lab_np = np.asarray(label)
    fp8 = ml_dtypes.float8_e4m3
    bf16 = ml_dtypes.bfloat16
    sel = np.eye(P, dtype=bf16)
    in_maps = []
    for k in range(N_CORES):
        xs = outputs[:, :, k * XS:(k + 1) * XS].reshape(B, C, P, F)
        xs = np.ascontiguousarray(xs.transpose(1, 2, 0, 3)).reshape(C, P, FB)
        ls = lab_np[:, k * XS:(k + 1) * XS].reshape(B, P, F)
        ls = np.ascontiguousarray(ls.transpose(1, 0, 2)).reshape(P, FB)
        in_maps.append({"x": xs.astype(fp8), "lab": ls.astype(bf16),
                        "sel": sel})
    res = run_bass_kernel_spmd(nc, in_maps, core_ids=list(range(N_CORES)))
    intersect = np.zeros(C, np.float64)
    sumsq = np.zeros(C, np.float64)
    for r in res.results:
        st = r["stats"].astype(np.float64)
        intersect += st[0].sum(axis=0)
        sumsq += st[1].sum(axis=0)
    labels_sum = np.bincount(
        lab_np.reshape(-1).astype(np.int64), minlength=C).astype(np.float64)
    dice = (2.0 * intersect + SMOOTH) / (sumsq + labels_sum + SMOOTH)
    return np.float32(np.mean(1.0 - dice))
